# revision 19
# baseline (speedup 1.0000x reference)
"""Trainium2 Bass kernel: 2-level 3D DWT forward (J=2, length-2 filters).

Math: with L=2 taps and even extents there is no padding, so one DWT level
is a separable 2x2x2 stride-2 block transform

    y[bw,bh,bd, d',h',w'] = sum_{i,j,k} F[bd,i] F[bh,j] F[bw,k]
                            * x[2d'+i, 2h'+j, 2w'+k],
    F = [[h0[0], h0[1]], [h1[0], h1[1]]],  band index b = bw*4 + bh*2 + bd.

Sharding: pure data parallel, one (n, c) 128^3 volume per core (N*C = 8).

Per-core pipeline (level 0, 128^3 -> 8 x 64^3):
  - DMA x into SBUF with partitions p = i*64 + j*32 + g, where (i, j) are the
    D/H parities and g = d' mod 32 (d' = 32q + g). Free dims (q, h', w).
  - PE: one matmul pass with a block-diagonal 128x128 stationary
    S[(i,j,g'), (b1,g)] = F[bd,i] F[bh,j] delta(g,g'), b1 = bh*2 + bd,
    fuses the D and H filter passes. PSUM out partitions = (b1, g).
  - DVE: W pass on stride-2 PSUM views:
        out = tap1*odd;  out = tap0*even + out   (in-place fused MAC)
    with the filter taps as runtime [128,1] scalar APs.
  - Outputs: whole levels go to y0[8,64^3] / y1[8,32^3]; the stationary's
    column order makes bands partition-contiguous, so each half-level is ONE
    DMA (lo tile -> y0[0:4], hi tile -> y0[4:8]). Level 1 re-reads y0[0].
    The host slices band 0 (ll / next level) vs bands 1..7 (yh outputs).

Hardware constraints honored here:
  - DMA access patterns are limited to 3 dims per side -> x is loaded by 8
    DMAs (one per (i, j, q)), level-1 input by 4 DMAs (one per (i2, j2)).
  - Engine instructions (PE matmul, DVE ops) carry at most ONE sync wait
    (fp32 matmuls are self-loading: no separate LDWEIGHTS slot). Tile only
    emits waits for sem ticks an engine has not yet observed, so tiny
    K=32/N=1 dummy matmuls into fresh columns of a dedicated dummy-PSUM
    bank pre-absorb, one wait each:
      * per input-DMA-lane waits (4 lanes per q-half of x, 4 for level 1),
      * PSUM slot-recycle deps: two "pre-absorbers" read the recycled
        group's lo/hi out-tiles (absorbing the DVE reader ticks), then a
        "slot-absorber" writes into the recycled tile (absorbing PE drain).
    On DVE, a leading wco-copy absorbs the coefficient DMA lane, out-tiles
    are never recycled (one slot per group), and the in-place MAC keeps
    each DVE op at a single chained dependency.
"""

import numpy as np
from contextlib import ExitStack

import concourse.bacc as bacc
import concourse.bass as bass
import concourse.mybir as mybir
import concourse.tile as tile
from concourse.bass_utils import run_bass_kernel_spmd

F32 = mybir.dt.float32
MUL = mybir.AluOpType.mult
ADD = mybir.AluOpType.add

_NC_CACHE: dict = {}


def _build_nc():
    nc = bacc.Bacc("TRN2", target_bir_lowering=False, debug=False)

    x_d = nc.dram_tensor("x", [128, 128, 128], F32, kind="ExternalInput").ap()
    stat_d = nc.dram_tensor("stat", [128, 128], F32, kind="ExternalInput").ap()
    wco_d = nc.dram_tensor("wco", [128, 4], F32, kind="ExternalInput").ap()

    y0_d = nc.dram_tensor("y0", [8, 64, 64, 64], F32, kind="ExternalOutput").ap()
    y1_d = nc.dram_tensor("y1", [8, 32, 32, 32], F32, kind="ExternalOutput").ap()

    # DRAM-side views.
    # x[d, h, w] with d = 2*(32q+g)+i, h = 2h'+j -> dims (i, j, g, q, h', w)
    x6 = x_d.rearrange("(q g i) (hp j) w -> i j g q hp w", q=2, g=32, i=2, hp=64, j=2)
    # y0[b, d', h', w'] with d' = 32q+g -> dims (b, g, q, h', w')
    y0v = y0_d.rearrange("b (q g) hp wp -> b g q hp wp", q=2, g=32)
    # level-1 load: y0[0][d2, h2, w2], d2 = 2g2+i2, h2 = 2h2'+j2
    y0r = y0_d[0].rearrange(
        "(g2 i2) (h2p j2) w2 -> i2 j2 g2 h2p w2", g2=32, i2=2, h2p=32, j2=2
    )

    with ExitStack() as ctx:
        tc = ctx.enter_context(tile.TileContext(nc))
        const_pool = ctx.enter_context(tc.tile_pool(name="const", bufs=1))
        xpool = ctx.enter_context(tc.tile_pool(name="xin", bufs=1))
        opool = ctx.enter_context(tc.tile_pool(name="out", bufs=16))
        o1pool = ctx.enter_context(tc.tile_pool(name="out1", bufs=1))
        ppool = ctx.enter_context(tc.tile_pool(name="psum", bufs=3, space="PSUM"))
        dpool = ctx.enter_context(tc.tile_pool(name="pdum", bufs=1, space="PSUM"))

        stat_sb = const_pool.tile([128, 128], F32, tag="stat")
        nc.sync.dma_start(out=stat_sb[:, :], in_=stat_d[:, :])
        wco_sb = const_pool.tile([128, 4], F32, tag="wco")
        nc.sync.dma_start(out=wco_sb[:, :], in_=wco_d[:, :])
        # First DVE op: absorb the wco DMA-lane wait alone.
        wabs = const_pool.tile([128, 4], F32, tag="wabs")
        nc.vector.tensor_copy(wabs[:, :], wco_sb[:, :])
        c_lo0 = wco_sb[:, 0:1]
        c_lo1 = wco_sb[:, 1:2]
        c_hi0 = wco_sb[:, 2:3]
        c_hi1 = wco_sb[:, 3:4]

        # dummy-PSUM bank: every dummy matmul writes its own fresh column
        pdum = dpool.tile([128, 512], F32, tag="pdum")
        dummy_col = [0]

        def dummy_mm(lhsT, rhs, tile_position=None):
            c = dummy_col[0]
            dummy_col[0] += 1
            return nc.tensor.matmul(
                pdum[0:1, c : c + 1], lhsT, rhs,
                start=True, stop=True, tile_position=tile_position,
            )

        # very first PE op: absorb the stat DMA-lane wait alone
        dummy_mm(stat_sb[0:32, 0:1], stat_sb[0:32, 1:2])

        # ---------------- level 0 ----------------
        # whole volume resident: [128 part = (i,j,g), 16384 free = (q,h',w)]
        # loaded by 8 DMAs (one per (i, j, q)): 3-dim src APs, 1 MiB each
        xt = xpool.tile([128, 16384], F32, tag="xin")
        for q in range(2):
            for i in range(2):
                for j in range(2):
                    nc.sync.dma_start(
                        out=xt[i * 64 + j * 32 : i * 64 + j * 32 + 32,
                               8192 * q : 8192 * (q + 1)],
                        in_=x6[i, j, :, q, :, :],
                    )

        prev_tiles = []  # (lo_t, hi_t) per group, for slot-recycle absorption
        PB = 3  # psum bufs

        def absorb_slot_recycle(P, n):
            if n < PB:
                return
            lo_p, hi_p = prev_tiles[n - PB]
            # pre-absorbers: observe the DVE ticks of the recycled group's
            # last readers (1 wait each)
            pre1 = dummy_mm(stat_sb[0:32, 0:1], lo_p[0:32, 0:1])
            pre2 = dummy_mm(stat_sb[0:32, 0:1], hi_p[0:32, 0:1])
            # slot-absorber: write into the recycled tile; carries only the
            # PE bank-drain wait now. Ordering edges pin it after the
            # pre-absorbers (the scheduler would otherwise hoist it).
            slot = nc.tensor.matmul(
                P[0:1, 2:3], stat_sb[0:32, 0:1], stat_sb[0:32, 1:2],
                start=True, stop=True,
            )
            tile.add_dep_helper(slot.ins, pre1.ins, sync=False,
                                reason="slot-absorber after pre-absorbers")
            tile.add_dep_helper(slot.ins, pre2.ins, sync=False,
                                reason="slot-absorber after pre-absorbers")

        ngroup = 0
        for q in range(2):
            for o in range(8):  # h' octet: h' in [8o, 8o+8)
                P = ppool.tile([128, 1024], F32, tag="ps")
                absorb_slot_recycle(P, ngroup)
                if o == 0:
                    # absorb this q-half's 4 input-DMA-lane waits, one each
                    for base in (0, 32, 64, 96):
                        dummy_mm(
                            stat_sb[base : base + 32, 0:1],
                            xt[base : base + 32, 8192 * q : 8192 * q + 1],
                            tile_position=(base, 0) if base == 96 else None,
                        )
                for k in range(2):
                    off = 8192 * q + 512 * (2 * o + k)
                    nc.tensor.matmul(
                        P[:, 512 * k : 512 * (k + 1)],
                        stat_sb[:, :],
                        xt[:, off : off + 512],
                        start=True,
                        stop=True,
                    )
                Pv = P[:, :].rearrange("p (hp w) -> p hp w", hp=8, w=128)
                Pe = Pv[:, :, 0::2]
                Po = Pv[:, :, 1::2]

                lo_t = opool.tile([128, 512], F32, tag="lo")
                hi_t = opool.tile([128, 512], F32, tag="hi")
                lo3 = lo_t[:, :].rearrange("p (hp wp) -> p hp wp", hp=8, wp=64)
                hi3 = hi_t[:, :].rearrange("p (hp wp) -> p hp wp", hp=8, wp=64)

                # out = tap1*odd, then in-place out = tap0*even + out
                # (in-place STT is safe: each element is read before written)
                nc.vector.tensor_scalar(lo3, Po, c_lo1, None, MUL)
                nc.vector.scalar_tensor_tensor(lo3, Pe, c_lo0, lo3, MUL, ADD)
                nc.vector.tensor_scalar(hi3, Po, c_hi1, None, MUL)
                nc.vector.scalar_tensor_tensor(hi3, Pe, c_hi0, hi3, MUL, ADD)

                # bands 0..3 (lo) and 4..7 (hi): one DMA per out tile
                nc.gpsimd.dma_start(
                    out=y0v[0:4, :, q, 8 * o : 8 * (o + 1), :], in_=lo_t[:, :]
                )
                nc.gpsimd.dma_start(
                    out=y0v[4:8, :, q, 8 * o : 8 * (o + 1), :], in_=hi_t[:, :]
                )
                prev_tiles.append((lo_t, hi_t))
                ngroup += 1

        # ---------------- level 1 ----------------
        x1 = xpool.tile([128, 2048], F32, tag="x1")
        for i2 in range(2):
            for j2 in range(2):
                nc.gpsimd.dma_start(
                    out=x1[i2 * 64 + j2 * 32 : i2 * 64 + j2 * 32 + 32, :],
                    in_=y0r[i2, j2, :, :, :],
                )

        u_lo = o1pool.tile([128, 1024], F32, tag="ulo")
        u_hi = o1pool.tile([128, 1024], F32, tag="uhi")

        first = True
        for half in range(2):  # h2' halves of 16
            P1 = ppool.tile([128, 1024], F32, tag="ps")
            absorb_slot_recycle(P1, ngroup)
            if first:
                # absorb the 4 level-1 input-DMA-lane waits
                for base in (0, 32, 64, 96):
                    dummy_mm(
                        stat_sb[base : base + 32, 0:1],
                        x1[base : base + 32, 0:1],
                        tile_position=(base, 0) if base == 96 else None,
                    )
                first = False
            for k in range(2):
                off = 1024 * half + 512 * k
                nc.tensor.matmul(
                    P1[:, 512 * k : 512 * (k + 1)],
                    stat_sb[:, :],
                    x1[:, off : off + 512],
                    start=True,
                    stop=True,
                )
            P1v = P1[:, :].rearrange("p (hp w) -> p hp w", hp=16, w=64)
            P1e = P1v[:, :, 0::2]
            P1o = P1v[:, :, 1::2]

            u_lo3 = u_lo[:, 512 * half : 512 * (half + 1)].rearrange(
                "p (hp wp) -> p hp wp", hp=16, wp=32
            )
            u_hi3 = u_hi[:, 512 * half : 512 * (half + 1)].rearrange(
                "p (hp wp) -> p hp wp", hp=16, wp=32
            )

            nc.vector.tensor_scalar(u_lo3, P1o, c_lo1, None, MUL)
            nc.vector.scalar_tensor_tensor(u_lo3, P1e, c_lo0, u_lo3, MUL, ADD)
            nc.vector.tensor_scalar(u_hi3, P1o, c_hi1, None, MUL)
            nc.vector.scalar_tensor_tensor(u_hi3, P1e, c_hi0, u_hi3, MUL, ADD)

            # u tiles are written once per half: track a fake group so the
            # recycle absorber can observe these DVE ticks too
            prev_tiles.append((u_lo, u_hi))
            ngroup += 1

        nc.gpsimd.dma_start(out=y1_d[0:4], in_=u_lo[:, :])
        nc.gpsimd.dma_start(out=y1_d[4:8], in_=u_hi[:, :])

    # bacc lowering: splits >1-wait instructions into EventSemaphore chains
    # (TRN2 allows 1 sync wait per instruction, 2 on InstEventSemaphore)
    nc.compile()
    return nc


def _get_nc():
    if "nc" not in _NC_CACHE:
        _NC_CACHE["nc"] = _build_nc()
    return _NC_CACHE["nc"]


def check_waits(nc, engine_limit=1):
    """Build-time check: every engine instruction must carry <= 1 sync wait."""
    bad = []
    for blk in nc.m.functions[0].blocks:
        for inst in blk.instructions:
            ty = type(inst).__name__
            si = inst.sync_info
            if si is None:
                continue
            nw = len(si.on_wait or [])
            if ty in ("InstMatmult", "InstTensorScalarPtr", "InstTensorCopy",
                      "InstActivation", "InstTensorTensor") and nw > engine_limit:
                bad.append((inst.name, ty,
                            [(w.ant_name, w.wait_value) for w in si.on_wait]))
    return bad


def _make_stationary(h0: np.ndarray, h1: np.ndarray) -> np.ndarray:
    F = np.stack([h0, h1]).astype(np.float32)  # F[band_bit, tap]
    S = np.zeros((128, 128), np.float32)
    g = np.arange(32)
    for i in (0, 1):
        for j in (0, 1):
            for bd in (0, 1):
                for bh in (0, 1):
                    b1 = bh * 2 + bd
                    S[i * 64 + j * 32 + g, b1 * 32 + g] = F[bd, i] * F[bh, j]
    return S


def kernel(x: np.ndarray, h0: np.ndarray, h1: np.ndarray):
    x = np.ascontiguousarray(x, dtype=np.float32)
    h0 = np.asarray(h0, dtype=np.float32).reshape(2)
    h1 = np.asarray(h1, dtype=np.float32).reshape(2)

    S = _make_stationary(h0, h1)
    wco = np.tile(
        np.array([h0[0], h0[1], h1[0], h1[1]], np.float32)[None, :], (128, 1)
    )

    nc = _get_nc()
    in_maps = []
    for c in range(8):
        n, ch = divmod(c, 4)
        in_maps.append(
            {"x": np.ascontiguousarray(x[n, ch]), "stat": S, "wco": wco}
        )
    res = run_bass_kernel_spmd(nc, in_maps, list(range(8)))
    outs = res.results

    ll1 = np.empty((2, 4, 32, 32, 32), np.float32)
    yh0 = np.empty((2, 4, 7, 64, 64, 64), np.float32)
    yh1 = np.empty((2, 4, 7, 32, 32, 32), np.float32)
    for c in range(8):
        n, ch = divmod(c, 4)
        ll1[n, ch] = outs[c]["y1"][0]
        yh0[n, ch] = outs[c]["y0"][1:]
        yh1[n, ch] = outs[c]["y1"][1:]
    return (ll1, yh0, yh1)
